# revision 58
# baseline (speedup 1.0000x reference)
"""CoDAConv2d Trainium2 kernel (8-core SPMD, data-parallel over batch x H-halves).

Reference computation (per pixel, per sample):
    raw[o]   = w_pred[o, :] @ x + b_pred[o]          o = p*16 + co, p in [0,72)
    act[co]  = sum_p patches[p] * raw[p*16+co]
    n2[co]   = sum_p raw[p*16+co]^2
    out[co]  = act[co] / (sqrt(n2[co]) + 1e-6)

Device reformulation (never materializes the [B,72,16,H,W] weightings):
    act[co]  = sum_cp V[(cp,co)] * x[cp] + T[co]
        V    = W2^T @ patches      (static 3x3 conv, K=72 contraction on PE)
        T    = Twk^T @ patches     (PSUM-accumulated with the selection reduce)
    n2[co]   = sum_j (Y[(j,co)] + m[(j,co)])^2 + delta[co]
        Y    = Grep^T @ xrep       (K=128 on the replicated x; Grep = G/16)
        +m   is the ACT Square bias; out = act * Rsqrt(n2 + delta) (eps folded)

Packing trick: the per-chunk 16-row act/norm reductions use per-chunk
selection weights selk[k]/twk[k] (128-col sliding windows into one
zero-padded buffer) whose output block is rows 32k:32k+32 of a full
128-partition PSUM tile, accumulated over the 4 chunks of a pack (start on
k=0, stop on k=3). The packed [128, CH] act/norm tiles then need no
SBUF-side packing copies; the Abs_reciprocal_sqrt + final multiply run
once per pack, and each pack's multiply is deferred behind the next
chunk's prod so it never blocks the selp matmul chain.

Everything ships as bf16 (pk is the dominant traffic on the serial DMA
track; measured rel err ~5e-3 vs the 2e-2 budget). Per-chunk input DMAs in
consumption order keep the pipeline fed from the first chunk; junk warmup
matmuls ramp the PE clock to 2.4 GHz during the DMA window and junk
activations preload both ACT function tables there too.
"""

import numpy as np
from contextlib import ExitStack

C_IN = 8
C_OUT = 16
PATCH = 72          # C_IN * 3 * 3
B = 4
H = W = 112
HALF = 56           # output rows per shard (2 shards per batch sample)
NPX = HALF * W      # 6272 output pixels per core
CH = 448            # chunk = 4 output rows (matmul N, <=512 fp32 / psum bank)
NCHUNK = NPX // CH  # 14
PACK = 4            # chunks per pack (PSUM accumulation group)
NPACKS = (NCHUNK + PACK - 1) // PACK
WB16 = 704          # bf16 weights: 128 w2 | 128 grep | 224 twbuf | 224 selbuf
NCORES = 8

_CACHE = {}


def _build_program():
    """Build + compile the per-core Bass program (same program on all cores)."""
    if "nc" in _CACHE:
        return _CACHE["nc"]
    import concourse.bacc as bacc
    import concourse.tile as tile
    from concourse import mybir

    f32 = mybir.dt.float32
    f32r = mybir.dt.float32r   # same bits; PE streams 1 cyc/col vs 4 for f32
    bf16 = mybir.dt.bfloat16
    AF = mybir.ActivationFunctionType

    nc = bacc.Bacc("TRN2", target_bir_lowering=False, debug=False,
                   num_devices=NCORES)
    pk_d = nc.declare_dram_parameter("pk", [NPACKS * 128, 2 * PACK * CH], bf16,
                                     isOutput=False)
    wb_d = nc.declare_dram_parameter("wb", [128, WB16], bf16, isOutput=False)
    wf_d = nc.declare_dram_parameter("wf", [128, 2], f32, isOutput=False)
    out_d = nc.declare_dram_parameter("out", [NPACKS * 128, CH], bf16,
                                      isOutput=True)

    with tile.TileContext(nc) as tc, ExitStack() as ctx:
        singles = ctx.enter_context(tc.tile_pool(name="singles", bufs=1))
        sb = ctx.enter_context(tc.tile_pool(name="sb", bufs=6))
        packsb = ctx.enter_context(tc.tile_pool(name="packsb", bufs=3))
        psv = ctx.enter_context(tc.tile_pool(name="psv", bufs=2, space="PSUM"))
        psy = ctx.enter_context(tc.tile_pool(name="psy", bufs=2, space="PSUM"))
        psa = ctx.enter_context(tc.tile_pool(name="psa", bufs=2, space="PSUM"))
        psn = ctx.enter_context(tc.tile_pool(name="psn", bufs=2, space="PSUM"))

        # matmul weights first, then the first chunk's pk columns so compute
        # starts early; the remaining pk stream follows on alternating queues
        wb_sb = singles.tile([128, WB16], bf16, name="wb")
        nc.sync.dma_start(out=wb_sb[:], in_=wb_d[:])
        wf_sb = singles.tile([128, 2], f32, name="wf")
        nc.sync.dma_start(out=wf_sb[:], in_=wf_d[:])

        # PE p-state warmup: the tensor engine only reaches 2.4 GHz after
        # ~3us of sustained work. Junk matmuls on a memset tile while the
        # input DMAs stream keep the real matmuls at full clock throughout.
        # The junk activations pull both ACT function-table loads into the
        # startup window (the Abs_reciprocal_sqrt load otherwise lands
        # mid-pipeline and stalls the pack-0 normalize chain by ~1.3us).
        junk = singles.tile([128, CH], bf16, name="junk")
        nc.gpsimd.memset(junk[:], 0)
        jact = singles.tile([128, 32], f32, name="jact")
        nc.scalar.activation(jact[:], junk[0:128, 0:32], AF.Square,
                             bias=0.0, scale=1.0)
        nc.scalar.activation(jact[:], junk[0:128, 0:32],
                             AF.Abs_reciprocal_sqrt, bias=0.0, scale=1.0)
        for w in range(8):
            pool, tg = (psv, "v") if w % 2 == 0 else (psy, "y")
            wup = pool.tile([128, CH], f32, tag=tg, name="wup")
            nc.tensor.matmul(wup[:], junk[0:128, 0:128], junk[:],
                             start=True, stop=True)

        w2_sb = wb_sb[0:PATCH, 0:128]
        gr_sb = wb_sb[0:128, 128:256]
        # twk/selk are 128-col sliding windows into zero-padded buffers:
        # shifting the window start by -32k lands bm/sel at output block 32k
        twk = [wb_sb[0:PATCH, 352 - 32 * k:480 - 32 * k]
               for k in range(PACK)]
        selk = [wb_sb[0:128, 576 - 32 * k:704 - 32 * k]
                for k in range(PACK)]
        mv_sb = wf_sb[0:128, 0:1]
        dv_sb = wf_sb[0:128, 1:2]

        # pk tiles: per pack [128, 8*CH]; chunk k of the pack owns the
        # column window [2*CH*k, 2*CH*(k+1)) = [xrep | patches]
        pk_t = [singles.tile([128, 2 * PACK * CH], bf16, tag=f"pk{p}",
                             name=f"pk{p}")
                for p in range(NPACKS)]
        # one DMA per chunk window, in consumption order: the serial DMA
        # track streams ~637ns/chunk while PE consumes ~935ns/chunk, so
        # per-chunk granularity keeps the pipeline fed from chunk 0 on
        for i in range(NCHUNK):
            p, k = i // PACK, i % PACK
            eng = nc.gpsimd if i % 2 == 0 else nc.sync
            eng.dma_start(out=pk_t[p][:, 2 * CH * k:2 * CH * (k + 1)],
                          in_=pk_d[128 * p:128 * (p + 1),
                                   2 * CH * k:2 * CH * (k + 1)])


        def flush(pend):
            p, a_ps, rns = pend
            out_sb = packsb.tile([128, CH], bf16, tag="out", name="out_sb")
            nc.vector.tensor_mul(out_sb[:], a_ps[:], rns[:])
            eng = nc.gpsimd if p % 2 == 0 else nc.sync
            eng.dma_start(out=out_d[128 * p:128 * (p + 1), :],
                          in_=out_sb[:])

        packs = {}
        pending = None
        for i in range(NCHUNK):
            p, k = i // PACK, i % PACK
            kp = min(PACK, NCHUNK - p * PACK)
            lo = 2 * CH * k
            xrep = pk_t[p][:, lo:lo + CH]
            patches = pk_t[p][0:PATCH, lo + CH:lo + 2 * CH]

            y_ps = psy.tile([128, CH], f32, tag="y")
            nc.tensor.matmul(y_ps[:], gr_sb, xrep, start=True, stop=True)
            v_ps = psv.tile([128, CH], f32, tag="v")
            nc.tensor.matmul(v_ps[:], w2_sb, patches, start=True, stop=True)

            if k == 0:
                packs[p] = (
                    psa.tile([128, CH], f32, tag="actp", name="act_pack"),
                    psn.tile([128, CH], f32, tag="nrmp", name="nrm_pack"),
                )
            a_ps, n_ps = packs[p]

            nc.tensor.matmul(a_ps[:], twk[k], patches,
                             start=(k == 0), stop=False)
            ysq = sb.tile([128, CH], bf16, tag="ysq")
            nc.scalar.activation(ysq[:], y_ps[:], AF.Square,
                                 bias=mv_sb, scale=1.0)
            prod = sb.tile([128, CH], bf16, tag="prod")
            nc.vector.tensor_mul(prod[:], v_ps[:], xrep)
            # the previous pack's final multiply runs on DVE behind this
            # chunk's prod, so the next selp never waits on the pack chain
            if pending is not None:
                flush(pending)
                pending = None
            last = k == kp - 1
            # on the pack's last chunk the n-side finishes first so the
            # ARsqrt overlaps the final act-side matmul
            nc.tensor.matmul(n_ps[:], selk[k], ysq[:],
                             start=(k == 0), stop=last)
            if last:
                rns = packsb.tile([128, CH], f32, tag="rns")
                nc.scalar.activation(rns[:], n_ps[:],
                                     AF.Abs_reciprocal_sqrt,
                                     bias=dv_sb, scale=1.0)
            nc.tensor.matmul(a_ps[:], selk[k], prod[:],
                             start=False, stop=last)
            if last:
                pending = (p, a_ps, rns)
        flush(pending)

    nc.compile()
    _CACHE["nc"] = nc
    return nc


def make_weights(w_pred, b_pred):
    """Host-side static weight prep -> (wb bf16 [128,1280]
    w2|grep|twk|selk, wf f32 [128,2] mv|dv)."""
    import ml_dtypes
    w_pred = np.asarray(w_pred, dtype=np.float64)
    b_pred = np.asarray(b_pred, dtype=np.float64)
    wr = w_pred.reshape(PATCH, C_OUT, C_IN)        # [p, co, c]
    bm = b_pred.reshape(PATCH, C_OUT)              # [p, co]
    w2 = np.ascontiguousarray(wr.transpose(0, 2, 1)).reshape(
        PATCH, C_IN * C_OUT)                       # [p, (cp,co)]
    A = np.einsum('poc,pod->ocd', wr, wr)          # [co, 8, 8]
    u = np.einsum('po,poc->oc', bm, wr)            # [co, 8]
    s = np.einsum('po,po->o', bm, bm)              # [co]
    L = np.linalg.cholesky(A)                      # [co, 8, 8]
    gq = L.transpose(1, 2, 0).reshape(C_IN, C_IN * C_OUT)  # [c, (j,co)]
    # Grep[(cp,co2), (j,co)] = G[cp, (j,co)] / 16  (sums over co2 to G @ x)
    grep = np.repeat(gq / C_OUT, C_OUT, axis=0)    # [128, 128]
    m = np.stack([np.linalg.solve(L[o], u[o]) for o in range(C_OUT)])  # [co, j]
    delta = s - (m * m).sum(1)                     # [co]

    wbm = np.zeros((128, WB16), dtype=np.float64)
    wbm[0:PATCH, 0:128] = w2
    wbm[0:128, 128:256] = grep
    sel = np.tile(np.eye(C_OUT), (C_IN, 1))                # (cp,co) -> co
    wbm[0:PATCH, 352:352 + C_OUT] = bm
    wbm[0:128, 576:576 + C_OUT] = sel

    wfm = np.zeros((128, 2), dtype=np.float64)
    wfm[:, 0] = m.T.reshape(128)                   # mv: (j,co) order
    dv = np.ones(128)
    for k in range(PACK):
        dv[32 * k:32 * k + C_OUT] = delta
    wfm[:, 1] = dv
    return (np.ascontiguousarray(wbm, dtype=ml_dtypes.bfloat16),
            np.ascontiguousarray(wfm, dtype=np.float32))


def make_shard_inputs(in_tensor, core):
    """Host prep for one core: pk [NPACKS*128, 8*CH] bf16, chunk windows
    [xrep | patches]."""
    import ml_dtypes
    b, sgn = core // 2, core % 2
    r0 = sgn * HALF
    pad = np.zeros((C_IN, H + 2, W + 2), dtype=np.float32)
    pad[:, 1:1 + H, 1:1 + W] = in_tensor[b]
    pat = np.empty((C_IN, 3, 3, HALF, W), dtype=np.float32)
    for di in range(3):
        for dj in range(3):
            pat[:, di, dj] = pad[:, r0 + di:r0 + di + HALF, dj:dj + W]
    pat = pat.reshape(PATCH, NPX)
    xin = in_tensor[b, :, r0:r0 + HALF, :].reshape(C_IN, NPX)
    xr16 = np.repeat(xin, C_OUT, axis=0)           # [(cp,co), n]
    pk = np.zeros((NPACKS, 128, 2 * PACK * CH), dtype=np.float32)
    for i in range(NCHUNK):
        p, k = i // PACK, i % PACK
        c0, c1 = i * CH, (i + 1) * CH
        lo = 2 * CH * k
        pk[p, :, lo:lo + CH] = xr16[:, c0:c1]
        pk[p, 0:PATCH, lo + CH:lo + 2 * CH] = pat[:, c0:c1]
    return np.ascontiguousarray(
        pk.reshape(NPACKS * 128, 2 * PACK * CH)).astype(ml_dtypes.bfloat16)


def unscramble(raw):
    """Device out [NPACKS*128, CH] bf16 -> [C_OUT, HALF, W] f32."""
    v = np.asarray(raw, dtype=np.float32)
    v = v.reshape(NPACKS * PACK, 32, CH)[:NCHUNK, :C_OUT, :]  # [i, co, j]
    v = v.transpose(1, 0, 2).reshape(C_OUT, NPX)
    return v.reshape(C_OUT, HALF, W)


def kernel(in_tensor, w_pred, b_pred):
    from concourse.bass_utils import run_bass_kernel_spmd

    in_tensor = np.asarray(in_tensor, dtype=np.float32)
    nc = _build_program()
    wb, wf = make_weights(w_pred, b_pred)
    in_maps = [{"pk": make_shard_inputs(in_tensor, c),
                "wb": wb, "wf": wf}
               for c in range(NCORES)]
    res = run_bass_kernel_spmd(nc, in_maps, list(range(NCORES)))
    out = np.empty((B, C_OUT, H, W), dtype=np.float32)
    for c in range(NCORES):
        b, sgn = c // 2, c % 2
        out[b, :, sgn * HALF:(sgn + 1) * HALF, :] = \
            unscramble(res.results[c]["out"])
    return out
